# revision 33
# baseline (speedup 1.0000x reference)
"""Trainium2 Bass kernel for nn_LowRankSVDBlock (8-core SPMD).

Sharding: data-parallel over batch (2 groups of 4 cores); within a group,
tensor-parallel over heads for attention (4 heads/core) and token-parallel
(512 tokens/core) after it. The out-projection stage-1 partial sums are
combined with 4 per-query-chunk ReduceScatters (bf16) that overlap the
attention compute; the receive side (transpose + out-proj stage 2 + LN2 +
fc1 stage 1) runs in two query-chunk pairs, the first pipelined underneath
the last attention chunk. The FFN runs full-width (512 tokens) at the end,
with gelu streaming behind fc1-stage-2 and fc2 following immediately.

Activations keep features on partitions / tokens on the free dim. Large
GEMMs run in fp8(e4m3) DoubleRow mode with power-of-2 weight prescales
folded back out later. Softmax uses an appended ones-column in V for the
denominator, a fast-approx reciprocal, and bf16 1xN->64xN PE broadcasts.
Both LayerNorms fold their gain into the next matmul's weights (LN2 via a
rank-space correction, so no explicit normalized tensor is materialized).
"""

import numpy as np
import ml_dtypes
from contextlib import ExitStack

import concourse.bass as bass
import concourse.tile as tile
from concourse import bacc, mybir
from concourse import bass_utils

BF16 = mybir.dt.bfloat16
F32 = mybir.dt.float32
FP8 = mybir.dt.float8e4
AF = mybir.ActivationFunctionType
ALU = mybir.AluOpType
DR = mybir.MatmulPerfMode.DoubleRow

B, S, D, H, DH = 2, 2048, 1024, 16, 64
R = 32          # attention rank
ROUT = 512      # out-proj rank
I = 4096        # ffn inner
RFC = 512       # fc rank
NCORE = 8
TOK = 512       # tokens per core (4 chunks of 128)
HPC = 4         # heads per core
LN_EPS = 1e-5
NF8 = ml_dtypes.float8_e4m3

_cache = {}


def _build_program():
    nc = bacc.Bacc("TRN2", target_bir_lowering=False, debug=False,
                   num_devices=NCORE)

    def din(name, shape, dt):
        return nc.dram_tensor(name, list(shape), dt, kind="ExternalInput")

    hb8 = din("hb8", (128, 8, S), FP8)         # hidden[b].T fp8
    hr = din("hr", (128, 8, TOK), F32)         # residual rows (+out_b)
    u8 = [din(f"u8{p}", (128, 8, 128), FP8) for p in range(3)]
    uc = [din(f"uc{p}", (128, 1), F32) for p in range(3)]
    v2 = [din(f"v2{p}", (128, 2, 128), BF16) for p in range(3)]
    b2q = din("b2q", (128, 2), F32)
    b2k = din("b2k", (128, 2), F32)
    bv128 = din("bv128", (128, 4, HPC * DH), F32)
    trineg = din("trineg", (128, 128), BF16)   # 0 if col>=row else -240
    ident = din("ident", (128, 128), BF16)     # identity (PE transpose)
    ouT8 = din("ouT8", (128, 2, ROUT), FP8)    # this core's out_U rows, x32
    ovT8 = din("ovT8", (128, 4, D), FP8)
    f1u8 = din("f1u8", (128, 8, RFC), FP8)     # ln2_g-folded fc1_U, x32
    f1c1 = din("f1c1", (128, 4), F32)          # sum_f f1u8 (dequant)
    f1c2 = din("f1c2", (128, 4), F32)          # fc1_U^T ln2_b
    f1v8 = din("f1v8", (128, 4, I), FP8)
    f2u8 = din("f2u8", (128, 32, RFC), FP8)
    f2v8 = din("f2v8", (128, 4, D), FP8)
    f1b = din("f1b", (128, 32), F32)
    f2b = din("f2b", (128, 8), F32)
    out_t = nc.dram_tensor("out_t", [128, 8, TOK], F32,
                           kind="ExternalOutput")

    RG = [[0, 1, 2, 3], [4, 5, 6, 7]]

    with tile.TileContext(nc) as tc, ExitStack() as top:
        wp = top.enter_context(tc.tile_pool(name="weights", bufs=1))
        cp = top.enter_context(tc.tile_pool(name="consts", bufs=1))
        dramp = top.enter_context(tc.tile_pool(name="dram", bufs=1,
                                               space="DRAM"))
        earlyA = ExitStack()
        wpA = earlyA.enter_context(tc.tile_pool(name="wpA", bufs=1))

        # ---------- input / weight DMAs (issued up front) ----------
        hb = wpA.tile([128, 8, S], FP8, name="hb")
        for tck in range(4):
            sl = slice(tck * 512, (tck + 1) * 512)
            nc.sync.dma_start(hb[:, :, sl], hb8[:, :, sl])
        u_sb = []
        for p in range(3):
            w = wpA.tile([128, 8, 128], FP8, name=f"u8{p}")
            nc.sync.dma_start(w[:], u8[p][:, :, :])
            u_sb.append(w)
        v2_sb = []
        for p in range(3):
            w = wpA.tile([128, 2, 128], BF16, name=f"v2{p}")
            nc.sync.dma_start(w[:], v2[p][:, :, :])
            v2_sb.append(w)
        uc_sb = []
        for p in range(3):
            w = cp.tile([128, 1], F32, name=f"uc{p}")
            nc.sync.dma_start(w[:], uc[p][:, :])
            uc_sb.append(w)
        b2q_sb = cp.tile([128, 2], F32, name="b2q")
        nc.sync.dma_start(b2q_sb[:], b2q[:, :])
        b2k_sb = cp.tile([128, 2], F32, name="b2k")
        nc.sync.dma_start(b2k_sb[:], b2k[:, :])
        bv_sb = cp.tile([128, 4, HPC * DH], F32, name="bv128")
        nc.sync.dma_start(bv_sb[:], bv128[:, :, :])
        tri_sb = cp.tile([128, 128], BF16, name="trineg")
        nc.sync.dma_start(tri_sb[:], trineg[:, :])
        id_sb = cp.tile([128, 128], BF16, name="ident")
        nc.sync.dma_start(id_sb[:], ident[:, :])
        ouT_sb = wp.tile([128, 2, ROUT], FP8, name="ouT8")
        nc.sync.dma_start(ouT_sb[:], ouT8[:, :, :])
        # late-phase weights (DMA streams while attention computes)
        ovT_sb = wp.tile([128, 4, D], FP8, name="ovT8")
        f1u_sb = wp.tile([128, 8, RFC], FP8, name="f1u8")
        f1c1_sb = cp.tile([128, 4], F32, name="f1c1")
        nc.sync.dma_start(f1c1_sb[:], f1c1[:, :])
        f1c2_sb = cp.tile([128, 4], F32, name="f1c2")
        nc.sync.dma_start(f1c2_sb[:], f1c2[:, :])
        hr_sb = wp.tile([128, 8, TOK], F32, name="hr")
        f1b_sb = cp.tile([128, 32], F32, name="f1b")
        f1v_sb = wp.tile([128, 4, I], FP8, name="f1v8")
        f2u_sb = wp.tile([128, 32, RFC], FP8, name="f2u8")
        f2v_sb = wp.tile([128, 4, D], FP8, name="f2v8")
        f2b_sb = cp.tile([128, 8], F32, name="f2b")

        # small constants
        ones2 = cp.tile([128, 2, DH], FP8, name="ones2")
        nc.vector.memset(ones2[:], 1.0)
        ones128b = cp.tile([1, 128], BF16, name="ones128b")
        nc.vector.memset(ones128b[:], 1.0)
        eights64 = cp.tile([1, DH], BF16, name="eights64")
        nc.vector.memset(eights64[:], 8.0)
        c32 = cp.tile([128, 1], F32, name="c32")
        nc.vector.memset(c32[:], 1.0 / 32.0)
        eps1 = cp.tile([1, 1], F32, name="eps1")
        nc.vector.memset(eps1[:], LN_EPS)

        # persistent activation tiles
        hT = wp.tile([128, 8, TOK], F32, name="hT")
        hbf = wp.tile([128, 8, TOK], FP8, name="hbf")
        QT = [wp.tile([128, S], BF16, name=f"QT{f}") for f in range(2)]
        KT = [wp.tile([128, S], BF16, name=f"KT{f}") for f in range(2)]
        VA = [wp.tile([128, 16, DH + 32], FP8, name=f"VA{h}")
              for h in range(HPC)]
        Yn = wp.tile([128, 2, S], FP8, name="Yn")
        rk = [wpA.tile([128, S], BF16, name=f"rk{p}") for p in range(3)]
        rbs = [wpA.tile([128, 512], BF16, name=f"rbs{t}") for t in range(4)]
        sbs = [wpA.tile([128, 512], BF16, name=f"sbs{t}") for t in range(4)]
        a1 = wp.tile([128, 4, TOK], FP8, name="a1")
        h1 = wp.tile([128, 32, TOK], FP8, name="h1")

        rs_in = [dramp.tile([4 * 128, 512], BF16, name=f"rs_in{q}")
                 for q in range(4)]
        rs_out = [dramp.tile([128, 512], BF16, name=f"rs_out{q}")
                  for q in range(4)]

        # first ACT instruction pins the sqrt table set for phase A
        warm = cp.tile([1, 1], F32, name="warm")
        nc.scalar.activation(warm[:], eps1[:], AF.Sqrt)

        # ---------- Phase A/B/C: LN1 stats + QKV (per 512-tok chunk) -----
        phA = ExitStack()
        sqp = phA.enter_context(tc.tile_pool(name="sq", bufs=2))
        stp = phA.enter_context(tc.tile_pool(name="lnsm", bufs=2))
        wps = phA.enter_context(
            tc.tile_pool(name="wps", bufs=1, space="PSUM"))
        bcps = phA.enter_context(
            tc.tile_pool(name="bcps", bufs=1, space="PSUM"))

        sq2t = {}

        def issue_sq(tck):
            sl = slice(tck * 512, (tck + 1) * 512)
            sq2 = sqp.tile([128, 8, 512], FP8, name="sq2")
            nc.scalar.activation(sq2[:], hb[:, :, sl], AF.Square)
            sq2t[tck] = sq2

        def issue_abc(tck):
            sl = slice(tck * 512, (tck + 1) * 512)
            # QKV stage 1 (PE, fp8 DR); no LN dependence
            s1t = []
            for p in range(3):
                ps = wps.tile([128, 512], F32, name="s1", bufs=3)
                for fc in range(0, 8, 2):
                    nc.tensor.matmul(ps[:], u_sb[p][:, fc:fc + 2, :],
                                     hb[:, fc:fc + 2, sl],
                                     start=(fc == 0), stop=(fc == 6),
                                     perf_mode=DR)
                s1t.append(ps)
            # LN1 stats: sum/sumsq matmuls, Sqrt + fast recip
            sq2 = sq2t.pop(tck)
            sum_ps = wps.tile([DH, 512], F32, name="s2", bufs=2)
            sq_ps = wps.tile([DH, 512], F32, name="s2", bufs=2)
            for fc in range(0, 8, 2):
                nc.tensor.matmul(sum_ps[:], ones2[:],
                                 hb[:, fc:fc + 2, sl],
                                 start=(fc == 0), stop=(fc == 6),
                                 perf_mode=DR)
                nc.tensor.matmul(sq_ps[:], ones2[:], sq2[:, fc:fc + 2, :],
                                 start=(fc == 0), stop=(fc == 6),
                                 perf_mode=DR)
            mu = stp.tile([1, 512], F32, name="mu")
            nc.vector.tensor_scalar(mu[:], sum_ps[0:1, :], 1.0 / D,
                                    None, ALU.mult)
            t1 = stp.tile([1, 512], F32, name="t1")
            nc.vector.tensor_mul(t1[:], sum_ps[0:1, :], mu[:])
            dvar = stp.tile([1, 512], F32, name="dvar")
            nc.vector.tensor_sub(dvar[:], sq_ps[0:1, :], t1[:])
            sdev = stp.tile([1, 512], F32, name="sdev")
            nc.scalar.activation(sdev[:], dvar[:], AF.Sqrt,
                                 bias=eps1[:], scale=1.0 / D)
            rr = stp.tile([1, 512], F32, name="rr")
            nc.vector.reciprocal_approx_fast(rr[:], sdev[:])
            rrow = stp.tile([1, 512], BF16, name="rrow")
            nc.vector.tensor_copy(rrow[:], rr[:])
            srow = stp.tile([1, 512], BF16, name="srow")
            nc.vector.tensor_mul(srow[:], mu[:], rr[:])
            rb_ps = bcps.tile([128, 512], F32, name="rb")
            nc.tensor.matmul(rb_ps[:], ones128b[:], rrow[:],
                             start=True, stop=True)
            nc.vector.tensor_copy(rbs[tck][:], rb_ps[:])
            sb_ps = bcps.tile([128, 512], F32, name="sb")
            nc.tensor.matmul(sb_ps[:], ones128b[:], srow[:],
                             start=True, stop=True)
            nc.vector.tensor_copy(sbs[tck][:], sb_ps[:])
            # LN correction onto the rank representation
            for p in range(3):
                t = stp.tile([128, 512], BF16, name="t")
                nc.vector.tensor_mul(t[:], s1t[p][:], rbs[tck][:])
                nc.vector.scalar_tensor_tensor(
                    rk[p][:, sl], sbs[tck][:], uc_sb[p][:], t[:],
                    ALU.mult, ALU.add)
            # QKV stage 2 (block-diag head pairs); Q/K bias on ACT
            for p, dst, bias in ((0, QT, b2q_sb), (1, KT, b2k_sb)):
                for f in range(2):
                    fp64 = slice(64 * f, 64 * f + 64)
                    ps = wps.tile([128, 512], F32, name="s2", bufs=2)
                    nc.tensor.matmul(ps[:], v2_sb[p][fp64, f, :],
                                     rk[p][fp64, sl],
                                     start=True, stop=True)
                    nc.scalar.activation(dst[f][:, sl], ps[:], AF.Identity,
                                         bias=bias[:, f:f + 1])
            for f in range(2):
                fp64 = slice(64 * f, 64 * f + 64)
                kt0 = tck * 4
                ps = wps.tile([128, 4, 128], F32, name="s2", bufs=2)
                for kk in range(4):
                    kt = kt0 + kk
                    nc.tensor.matmul(
                        ps[:, kk, :], rk[2][fp64, kt * 128:(kt + 1) * 128],
                        v2_sb[2][fp64, f, :], start=True, stop=True)
                for j in range(2):
                    h = 2 * f + j
                    nc.vector.tensor_tensor(
                        VA[h][:, kt0:kt0 + 4, 0:DH],
                        ps[:, :, j * DH:(j + 1) * DH],
                        bv_sb[:, :, h * DH:h * DH + DH], ALU.add)

        for h in range(HPC):
            nc.vector.memset(VA[h][:, :, DH:DH + 1], 8.0)
            nc.vector.memset(VA[h][:, :, DH + 1:DH + 32], 0.0)
        issue_sq(0)
        issue_sq(1)
        issue_abc(0)
        nc.sync.dma_start(ovT_sb[:], ovT8[:, :, :])
        nc.sync.dma_start(f1u_sb[:], f1u8[:, :, :])
        nc.sync.dma_start(hr_sb[:], hr[:, :, :])
        nc.sync.dma_start(f1b_sb[:], f1b[:, :])
        issue_sq(2)
        issue_abc(1)
        nc.sync.dma_start(f1v_sb[:], f1v8[:, :, :])
        nc.sync.dma_start(f2u_sb[:], f2u8[:, :, :])
        issue_sq(3)
        issue_abc(2)
        nc.sync.dma_start(f2v_sb[:], f2v8[:, :, :])
        nc.sync.dma_start(f2b_sb[:], f2b[:, :])
        issue_abc(3)

        phA.close()
        earlyA.close()

        # ---------- attention + per-qc RS + paired local out/LN2 ------
        phD = ExitStack()      # whole tail phase
        mps = phD.enter_context(
            tc.tile_pool(name="mps", bufs=2, space="PSUM"))
        ptp = phD.enter_context(tc.tile_pool(name="pt", bufs=10))
        epi = phD.enter_context(tc.tile_pool(name="epi", bufs=2))
        stp2 = phD.enter_context(tc.tile_pool(name="st", bufs=3))
        o1p = phD.enter_context(tc.tile_pool(name="o1p", bufs=2))
        rsp = phD.enter_context(tc.tile_pool(name="rsp", bufs=4))
        gp = phD.enter_context(tc.tile_pool(name="gp", bufs=2))
        phAt = ExitStack()     # attention-only psum pools
        scps = phAt.enter_context(
            tc.tile_pool(name="scps", bufs=2, space="PSUM"))
        pvps = phAt.enter_context(
            tc.tile_pool(name="pvps", bufs=2, space="PSUM"))

        def issue_head(h, qc):
            """scores (streamed), exp, then PVs for one (head, q-chunk).

            All score matmuls are issued before any PV so the PE never
            stalls on the exp chain mid-head (keeps the HAM clock warm)."""
            rho, f = h % 2, h // 2
            pp = slice(64 * rho, 64 * rho + 64)
            q0 = qc * 512
            ntile = 4 * qc + 4
            pv = pvps.tile([DH + 32, 512], F32, name="pv")
            pts = []
            for t0 in range(0, ntile, 2):
                diag = t0 >= 4 * qc
                sc = scps.tile([128, 2, 512], F32, name="sc")
                pt = ptp.tile([128, 2, 512], FP8, name="p8")
                if not diag:
                    for j in range(2):
                        nc.tensor.matmul(
                            sc[:, j, :],
                            KT[f][pp, (t0 + j) * 128:(t0 + j + 1) * 128],
                            QT[f][pp, q0:q0 + 512],
                            start=True, stop=True)
                    nc.scalar.activation(pt[:], sc[:], AF.Exp,
                                         scale=0.125)
                else:
                    for j in range(2):
                        c0 = (t0 + j - 4 * qc) * 128
                        nc.tensor.matmul(
                            sc[:, j, c0:],
                            KT[f][pp, (t0 + j) * 128:(t0 + j + 1) * 128],
                            QT[f][pp, q0 + c0:q0 + 512],
                            start=True, stop=True)
                        nc.vector.tensor_tensor(
                            sc[:, j, c0:c0 + 128], sc[:, j, c0:c0 + 128],
                            tri_sb[:], ALU.add)
                        if c0 > 0:
                            nc.vector.memset(pt[:, j, 0:c0], 0.0)
                        nc.scalar.activation(pt[:, j, c0:],
                                             sc[:, j, c0:], AF.Exp,
                                             scale=0.125)
                pts.append(pt)
            for i, pt in enumerate(pts):
                t0 = 2 * i
                nc.tensor.matmul(pv[:], VA[h][:, t0:t0 + 2, :],
                                 pt[:], start=(t0 == 0),
                                 stop=(t0 == ntile - 2), perf_mode=DR)
            return pv

        def issue_epilogue(h, qc, pv):
            rho, f = h % 2, h // 2
            q0 = qc * 512
            den = epi.tile([1, 512], F32, name="den")
            nc.vector.tensor_copy(den[:], pv[DH:DH + 1, :])
            rec = epi.tile([1, 512], F32, name="rec")
            nc.vector.reciprocal_approx_fast(rec[:], den[:])
            rec8 = epi.tile([1, 512], BF16, name="rec8")
            nc.gpsimd.tensor_copy(rec8[:], rec[:])
            rb_ps = mps.tile([DH, 512], F32, name="m")
            nc.tensor.matmul(rb_ps[:], eights64[:], rec8[:],
                             start=True, stop=True)
            rb_sb = epi.tile([DH, 512], BF16, name="rbsb")
            nc.vector.tensor_copy(rb_sb[:], rb_ps[:])
            nc.vector.tensor_mul(Yn[64 * rho:64 * rho + 64, f,
                                    q0:q0 + 512],
                                 pv[0:DH, :], rb_sb[:])

        def issue_attn(qc):
            pend = None
            for h in range(HPC):
                pv = issue_head(h, qc)
                if pend is not None:
                    issue_epilogue(h - 1, qc, pend)
                pend = pv
            issue_epilogue(HPC - 1, qc, pend)

        def issue_rs(qc):
            """out-proj stage-1 partials ([tok, rank]) + ReduceScatter."""
            for tb in range(4):
                t0 = qc * 512 + tb * 128
                ps = mps.tile([128, 512], F32, name="m")
                nc.tensor.matmul(
                    ps[:], Yn[:, :, t0:t0 + 128], ouT_sb[:, :, :],
                    start=True, stop=True, perf_mode=DR)
                st = rsp.tile([128, 512], BF16, name="st")
                nc.vector.tensor_copy(st[:], ps[:])
                nc.sync.dma_start(
                    rs_in[qc][tb * 128:(tb + 1) * 128, :], st[:])
            nc.gpsimd.collective_compute(
                "ReduceScatter", ALU.add, replica_groups=RG,
                ins=[rs_in[qc].opt()], outs=[rs_out[qc].opt()])

        def issue_down_pair(pr):
            """out-proj s2 + residual + LN2 + fc1 s1 for 2 query chunks."""
            tsl = slice(pr * 256, (pr + 1) * 256)
            ob = o1p.tile([128, 4, 256], FP8, name="o1")
            for half in range(2):
                qc = 2 * pr + half
                g = gp.tile([128, 512], BF16, name="g")
                nc.sync.dma_start(g[:], rs_out[qc][:, :])
                for rc in range(4):
                    tp = mps.tile([128, 128], BF16, name="m")
                    nc.tensor.matmul(tp[:], g[:, rc * 128:(rc + 1) * 128],
                                     id_sb[:], start=True, stop=True,
                                     is_transpose=True)
                    nc.vector.tensor_scalar(
                        ob[:, rc, half * 128:(half + 1) * 128], tp[:],
                        1.0 / 256.0, None, ALU.mult)
            # out-proj stage 2 + residual -> hT, hbf
            for g4 in range(4):
                o2ps = mps.tile([128, 2, 256], F32, name="m")
                for k in range(2):
                    ft = g4 * 2 + k
                    for rc in range(0, 4, 2):
                        nc.tensor.matmul(
                            o2ps[:, k, :],
                            ovT_sb[:, rc:rc + 2, ft * 128:(ft + 1) * 128],
                            ob[:, rc:rc + 2, :],
                            start=(rc == 0), stop=(rc == 2), perf_mode=DR)
                nc.vector.scalar_tensor_tensor(
                    hT[:, g4 * 2:g4 * 2 + 2, tsl], o2ps[:], c32[:],
                    hr_sb[:, g4 * 2:g4 * 2 + 2, tsl],
                    ALU.mult, ALU.add)
            nc.vector.tensor_copy(hbf[:, :, tsl], hT[:, :, tsl])
            sq8 = stp2.tile([128, 8, 256], FP8, name="sq8")
            nc.scalar.activation(sq8[:], hbf[:, :, tsl], AF.Square)
            # LN2 stats + rstd chain
            sms = mps.tile([DH, 2, 256], F32, name="m")
            for fc in range(0, 8, 2):
                nc.tensor.matmul(sms[:, 0, :], ones2[:],
                                 hbf[:, fc:fc + 2, tsl],
                                 start=(fc == 0), stop=(fc == 6),
                                 perf_mode=DR)
                nc.tensor.matmul(sms[:, 1, :], ones2[:],
                                 sq8[:, fc:fc + 2, :],
                                 start=(fc == 0), stop=(fc == 6),
                                 perf_mode=DR)
            mu = stp2.tile([1, 256], F32, name="mu2")
            nc.vector.tensor_scalar(mu[:], sms[0:1, 0, :], 1.0 / D,
                                    None, ALU.mult)
            t1 = stp2.tile([1, 256], F32, name="t12")
            nc.vector.tensor_mul(t1[:], sms[0:1, 0, :], mu[:])
            dvar = stp2.tile([1, 256], F32, name="dvar2")
            nc.vector.tensor_sub(dvar[:], sms[0:1, 1, :], t1[:])
            sdev = stp2.tile([1, 256], F32, name="sdev2")
            nc.scalar.activation(sdev[:], dvar[:], AF.Sqrt,
                                 bias=eps1[:], scale=1.0 / D)
            rr = stp2.tile([1, 256], F32, name="rr2")
            nc.vector.reciprocal_approx_fast(rr[:], sdev[:])
            rrow = stp2.tile([1, 256], BF16, name="rrow2")
            nc.vector.tensor_copy(rrow[:], rr[:])
            srow = stp2.tile([1, 256], BF16, name="srow2")
            nc.vector.tensor_mul(srow[:], mu[:], rr[:])
            bc = mps.tile([128, 2, 256], F32, name="m")
            nc.tensor.matmul(bc[:, 0, :], ones128b[:], rrow[:],
                             start=True, stop=True)
            nc.tensor.matmul(bc[:, 1, :], ones128b[:], srow[:],
                             start=True, stop=True)
            rbs2 = stp2.tile([128, 2, 256], BF16, name="rbs2")
            nc.vector.tensor_copy(rbs2[:], bc[:])
            # fc1 stage 1 on raw hbf + folded LN2 correction
            for rg in range(2):
                f1ps = mps.tile([128, 2, 256], F32, name="m")
                for k in range(2):
                    rc = rg * 2 + k
                    for fc in range(0, 8, 2):
                        nc.tensor.matmul(
                            f1ps[:, k, :],
                            f1u_sb[:, fc:fc + 2, rc * 128:(rc + 1) * 128],
                            hbf[:, fc:fc + 2, tsl],
                            start=(fc == 0), stop=(fc == 6), perf_mode=DR)
                for k in range(2):
                    rc = rg * 2 + k
                    t = stp2.tile([128, 256], BF16, name="tf1")
                    nc.vector.tensor_mul(t[:], f1ps[:, k, :], rbs2[:, 0, :])
                    u = stp2.tile([128, 256], F32, name="uf1")
                    nc.vector.scalar_tensor_tensor(
                        u[:], rbs2[:, 1, :], f1c1_sb[:, rc:rc + 1], t[:],
                        ALU.mult, ALU.subtract)
                    nc.vector.tensor_scalar(
                        a1[:, rc, tsl], u[:], -1.0 / 32.0,
                        f1c2_sb[:, rc:rc + 1], ALU.mult, ALU.add)

        ffn_pools = {}

        def open_ffn_pools():
            ffn_pools["f1"] = phD.enter_context(
                tc.tile_pool(name="f1ps2", bufs=2, space="PSUM"))
            ffn_pools["acc"] = phD.enter_context(
                tc.tile_pool(name="f2acc", bufs=2, space="PSUM"))
            ffn_pools["o"] = phD.enter_context(
                tc.tile_pool(name="f2o", bufs=1, space="PSUM"))
            ffn_pools["out"] = phD.enter_context(
                tc.tile_pool(name="outp", bufs=2))

        def issue_fc1_half(half):
            """fc1 s2 + gelu for one 256-token half (fills the RS window)."""
            hsl = slice(half * 256, half * 256 + 256)
            for it in range(32):
                ps = ffn_pools["f1"].tile([128, 256], F32, name="f1s2")
                for rc in range(0, 4, 2):
                    nc.tensor.matmul(
                        ps[:], f1v_sb[:, rc:rc + 2, it * 128:(it + 1) * 128],
                        a1[:, rc:rc + 2, hsl], start=(rc == 0),
                        stop=(rc == 2), perf_mode=DR)
                nc.scalar.activation(h1[:, it, hsl], ps[:], AF.Gelu,
                                     bias=f1b_sb[:, it:it + 1],
                                     scale=1.0 / 16.0)

        def issue_ffn():
            """fc2 (full width) + residual + output."""
            a2 = ffn_pools["out"].tile([128, 4, TOK], FP8, name="a2")
            for rt in range(4):
                acc = ffn_pools["acc"].tile([128, TOK], F32, name="acc")
                for ic in range(0, 32, 2):
                    nc.tensor.matmul(
                        acc[:], f2u_sb[:, ic:ic + 2, rt * 128:(rt + 1) * 128],
                        h1[:, ic:ic + 2, :], start=(ic == 0),
                        stop=(ic == 30), perf_mode=DR)
                nc.vector.tensor_scalar(a2[:, rt, :], acc[:], 1.0 / 32.0,
                                        None, ALU.mult)
            for g4 in range(4):
                ops = ffn_pools["o"].tile([128, 2, TOK], F32, name="f2o")
                for k in range(2):
                    ft = g4 * 2 + k
                    for rc in range(0, 4, 2):
                        nc.tensor.matmul(
                            ops[:, k, :],
                            f2v_sb[:, rc:rc + 2, ft * 128:(ft + 1) * 128],
                            a2[:, rc:rc + 2, :],
                            start=(rc == 0), stop=(rc == 2), perf_mode=DR)
                ot = ffn_pools["out"].tile([128, 2, TOK], F32, name="o")
                for k in range(2):
                    ft = g4 * 2 + k
                    nc.vector.tensor_scalar(ot[:, k, :], ops[:, k, :],
                                            c32[:], f2b_sb[:, ft:ft + 1],
                                            ALU.mult, ALU.add)
                nc.vector.tensor_tensor(
                    ot[:], ot[:], hT[:, g4 * 2:g4 * 2 + 2, :], ALU.add)
                nc.sync.dma_start(out_t[:, g4 * 2:g4 * 2 + 2, :], ot[:])

        issue_attn(0)
        issue_rs(0)
        issue_attn(1)
        issue_rs(1)
        issue_attn(2)
        issue_rs(2)
        issue_attn(3)
        issue_down_pair(0)
        issue_rs(3)
        phAt.close()
        open_ffn_pools()
        issue_fc1_half(0)
        issue_down_pair(1)
        issue_fc1_half(1)
        issue_ffn()
        phD.close()

    nc.compile()
    return nc


def _q8(x, scale):
    return np.clip(np.asarray(x, np.float32) * scale,
                   -448.0, 448.0).astype(NF8)


def _chunk(x, nch):
    """[nch*128, M] -> [128, nch, M] partition-major layout."""
    m = x.shape[1]
    return np.ascontiguousarray(
        x.reshape(nch, 128, m).transpose(1, 0, 2))


def _prep_inputs(inputs):
    bf = ml_dtypes.bfloat16
    hs = np.asarray(inputs["hidden_states"], np.float32)
    g1 = np.asarray(inputs["ln1_g"], np.float32)
    b1 = np.asarray(inputs["ln1_b"], np.float32)
    g1s = np.where(g1 == 0.0, 1.0, g1)
    g2 = np.asarray(inputs["ln2_g"], np.float32)
    b2 = np.asarray(inputs["ln2_b"], np.float32)

    tri = np.where(np.triu(np.ones((128, 128), np.float32)) > 0, 0.0,
                   -240.0).astype(bf)
    oU = np.asarray(inputs["out_U"], np.float32)

    f1U = np.asarray(inputs["fc1_U"], np.float32)
    f1u_eff = _q8(f1U * g2[:, None], 32.0)
    f1c1 = f1u_eff.astype(np.float32).sum(0)            # [512]
    f1c2 = (f1U.T @ b2)                                  # [512]

    shared = {
        "trineg": tri,
        "ident": np.eye(128, dtype=np.float32).astype(bf),
        "f1b": np.ascontiguousarray(
            np.asarray(inputs["fc1_b"], np.float32).reshape(32, 128).T),
        "f2b": np.ascontiguousarray(
            np.asarray(inputs["fc2_b"], np.float32).reshape(8, 128).T),
        "ovT8": _chunk(_q8(inputs["out_V"], 32.0), 4),
        "f1u8": _chunk(f1u_eff, 8),
        "f1c1": np.ascontiguousarray(f1c1.reshape(4, 128).T),
        "f1c2": np.ascontiguousarray(f1c2.reshape(4, 128).T),
        "f1v8": _chunk(_q8(inputs["fc1_V"], 16.0), 4),
        "f2u8": _chunk(_q8(inputs["fc2_U"], 32.0), 32),
        "f2v8": _chunk(_q8(inputs["fc2_V"], 32.0), 4),
    }

    qU, kU, vU = (np.asarray(inputs[k], np.float32)
                  for k in ("q_U", "k_U", "v_U"))
    qV, kV, vV = (np.asarray(inputs[k], np.float32)
                  for k in ("q_V", "k_V", "v_V"))
    qb, kb, vb = (np.asarray(inputs[k], np.float32)
                  for k in ("q_b", "k_b", "v_b"))
    ob = np.asarray(inputs["out_b"], np.float32)

    in_maps = []
    for c in range(NCORE):
        b, g = c // 4, c % 4
        hsel = slice(4 * g, 4 * g + 4)
        m = dict(shared)
        m["hb8"] = _chunk(_q8(hs[b].T, 1.0), 8)
        toks = np.concatenate(
            [np.arange(qc * 512 + g * 128, qc * 512 + g * 128 + 128)
             for qc in range(4)])
        hrm = hs[b, toks, :].T + ob[:, None]
        m["hr"] = _chunk(hrm, 8).astype(np.float32)
        for p, (U, V, bias) in enumerate(((qU, qV, qb), (kU, kV, kb),
                                          (vU, vV, vb))):
            ue = U[:, hsel, :].reshape(D, HPC * R) * g1[:, None]
            ue_q = _q8(ue, 32.0)
            m[f"u8{p}"] = _chunk(ue_q, 8)
            ue_f = ue_q.astype(np.float32)
            m[f"uc{p}"] = np.ascontiguousarray(
                -ue_f.sum(0)[:, None]).astype(np.float32)
            Vh = V[hsel]                              # [HPC, R, DH]
            scl = (8.0 / 32.0) if p == 2 else (1.0 / 32.0)
            bd = np.zeros((128, 2, 128), np.float32)
            for f in range(2):
                bd[64 * f:64 * f + 32, f, 0:DH] = Vh[2 * f] * scl
                bd[64 * f + 32:64 * f + 64, f, DH:2 * DH] = \
                    Vh[2 * f + 1] * scl
            m[f"v2{p}"] = bd.astype(bf)
            Ut = ue_f.reshape(D, HPC, R) / (32.0 * g1s[:, None, None])
            bcor = np.einsum('d,dhr,hre->he', b1, Ut, Vh) + bias[hsel]
            if p < 2:
                b2p = np.zeros((128, 2), np.float32)
                for f in range(2):
                    b2p[0:64, f] = bcor[2 * f]
                    b2p[64:128, f] = bcor[2 * f + 1]
                m["b2q" if p == 0 else "b2k"] = b2p
            else:
                bvb = np.broadcast_to((8.0 * bcor).reshape(1, 1, HPC * DH),
                                      (128, 4, HPC * DH))
                m["bv128"] = np.ascontiguousarray(bvb).astype(np.float32)
        m["ouT8"] = _chunk(_q8(oU[256 * g:256 * (g + 1), :], 32.0), 2)
        in_maps.append(m)
    return in_maps


def kernel(trace=False, tmpdir=None, **inputs):
    if "nc" not in _cache:
        _cache["nc"] = _build_program()
    nc = _cache["nc"]
    in_maps = _prep_inputs(inputs)
    res = bass_utils.run_bass_kernel_spmd(
        nc, in_maps, core_ids=list(range(NCORE)), trace=trace,
        tmpdir=tmpdir)
    out = np.zeros((B, S, D), np.float32)
    for c in range(NCORE):
        b, g = c // 4, c % 4
        toks = np.concatenate(
            [np.arange(qc * 512 + g * 128, qc * 512 + g * 128 + 128)
             for qc in range(4)])
        r = res.results[c]["out_t"]          # [128, 8, TOK]
        out[b, toks, :] = r.transpose(1, 0, 2).reshape(D, TOK).T
    if trace:
        return out, res
    return out


# revision 34
# speedup vs baseline: 1.0024x; 1.0024x over previous
"""Trainium2 Bass kernel for nn_LowRankSVDBlock (8-core SPMD).

Sharding: data-parallel over batch (2 groups of 4 cores); within a group,
tensor-parallel over heads for attention (4 heads/core) and token-parallel
(512 tokens/core) after it. The out-projection stage-1 partial sums are
combined with 4 per-query-chunk ReduceScatters (bf16) that overlap the
attention compute; the receive side (transpose + out-proj stage 2 + LN2 +
fc1 stage 1) runs in two query-chunk pairs, the first pipelined underneath
the last attention chunk. The FFN runs full-width (512 tokens) at the end,
with gelu streaming behind fc1-stage-2 and fc2 following immediately.

Activations keep features on partitions / tokens on the free dim. Large
GEMMs run in fp8(e4m3) DoubleRow mode with power-of-2 weight prescales
folded back out later. Softmax uses an appended ones-column in V for the
denominator, a fast-approx reciprocal, and bf16 1xN->64xN PE broadcasts.
Both LayerNorms fold their gain into the next matmul's weights (LN2 via a
rank-space correction, so no explicit normalized tensor is materialized).
"""

import numpy as np
import ml_dtypes
from contextlib import ExitStack

import concourse.bass as bass
import concourse.tile as tile
from concourse import bacc, mybir
from concourse import bass_utils

BF16 = mybir.dt.bfloat16
F32 = mybir.dt.float32
FP8 = mybir.dt.float8e4
AF = mybir.ActivationFunctionType
ALU = mybir.AluOpType
DR = mybir.MatmulPerfMode.DoubleRow

B, S, D, H, DH = 2, 2048, 1024, 16, 64
R = 32          # attention rank
ROUT = 512      # out-proj rank
I = 4096        # ffn inner
RFC = 512       # fc rank
NCORE = 8
TOK = 512       # tokens per core (4 chunks of 128)
HPC = 4         # heads per core
LN_EPS = 1e-5
NF8 = ml_dtypes.float8_e4m3

_cache = {}


def _build_program():
    nc = bacc.Bacc("TRN2", target_bir_lowering=False, debug=False,
                   num_devices=NCORE)

    def din(name, shape, dt):
        return nc.dram_tensor(name, list(shape), dt, kind="ExternalInput")

    hb8 = din("hb8", (128, 8, S), FP8)         # hidden[b].T fp8
    hr = din("hr", (128, 8, TOK), F32)         # residual rows (+out_b)
    u8 = [din(f"u8{p}", (128, 8, 128), FP8) for p in range(3)]
    uc = [din(f"uc{p}", (128, 1), F32) for p in range(3)]
    v2 = [din(f"v2{p}", (128, 2, 128), BF16) for p in range(3)]
    b2q = din("b2q", (128, 2), F32)
    b2k = din("b2k", (128, 2), F32)
    bv128 = din("bv128", (128, 4, HPC * DH), F32)
    trineg = din("trineg", (128, 128), BF16)   # 0 if col>=row else -240
    ident = din("ident", (128, 128), BF16)     # identity (PE transpose)
    ouT8 = din("ouT8", (128, 2, ROUT), FP8)    # this core's out_U rows, x32
    ovT8 = din("ovT8", (128, 4, D), FP8)
    f1u8 = din("f1u8", (128, 8, RFC), FP8)     # ln2_g-folded fc1_U, x32
    f1c1 = din("f1c1", (128, 4), F32)          # sum_f f1u8 (dequant)
    f1c2 = din("f1c2", (128, 4), F32)          # fc1_U^T ln2_b
    f1v8 = din("f1v8", (128, 4, I), FP8)
    f2u8 = din("f2u8", (128, 32, RFC), FP8)
    f2v8 = din("f2v8", (128, 4, D), FP8)
    f1b = din("f1b", (128, 32), F32)
    f2b = din("f2b", (128, 8), F32)
    out_t = nc.dram_tensor("out_t", [128, 8, TOK], F32,
                           kind="ExternalOutput")

    RG = [[0, 1, 2, 3], [4, 5, 6, 7]]

    with tile.TileContext(nc) as tc, ExitStack() as top:
        wp = top.enter_context(tc.tile_pool(name="weights", bufs=1))
        cp = top.enter_context(tc.tile_pool(name="consts", bufs=1))
        dramp = top.enter_context(tc.tile_pool(name="dram", bufs=1,
                                               space="DRAM"))
        earlyA = ExitStack()
        wpA = earlyA.enter_context(tc.tile_pool(name="wpA", bufs=1))

        # ---------- input / weight DMAs (issued up front) ----------
        hb = wpA.tile([128, 8, S], FP8, name="hb")
        for tck in range(4):
            sl = slice(tck * 512, (tck + 1) * 512)
            nsplit = 4 if tck == 0 else 2
            for fh in range(nsplit):
                w = 8 // nsplit
                fs = slice(fh * w, (fh + 1) * w)
                nc.sync.dma_start(hb[:, fs, sl], hb8[:, fs, sl])
        u_sb = []
        for p in range(3):
            w = wpA.tile([128, 8, 128], FP8, name=f"u8{p}")
            nc.sync.dma_start(w[:], u8[p][:, :, :])
            u_sb.append(w)
        v2_sb = []
        for p in range(3):
            w = wpA.tile([128, 2, 128], BF16, name=f"v2{p}")
            nc.sync.dma_start(w[:], v2[p][:, :, :])
            v2_sb.append(w)
        uc_sb = []
        for p in range(3):
            w = cp.tile([128, 1], F32, name=f"uc{p}")
            nc.sync.dma_start(w[:], uc[p][:, :])
            uc_sb.append(w)
        b2q_sb = cp.tile([128, 2], F32, name="b2q")
        nc.sync.dma_start(b2q_sb[:], b2q[:, :])
        b2k_sb = cp.tile([128, 2], F32, name="b2k")
        nc.sync.dma_start(b2k_sb[:], b2k[:, :])
        bv_sb = cp.tile([128, 4, HPC * DH], F32, name="bv128")
        nc.sync.dma_start(bv_sb[:], bv128[:, :, :])
        tri_sb = cp.tile([128, 128], BF16, name="trineg")
        nc.sync.dma_start(tri_sb[:], trineg[:, :])
        id_sb = cp.tile([128, 128], BF16, name="ident")
        nc.sync.dma_start(id_sb[:], ident[:, :])
        ouT_sb = wp.tile([128, 2, ROUT], FP8, name="ouT8")
        nc.sync.dma_start(ouT_sb[:], ouT8[:, :, :])
        # late-phase weights (DMA streams while attention computes)
        ovT_sb = wp.tile([128, 4, D], FP8, name="ovT8")
        f1u_sb = wp.tile([128, 8, RFC], FP8, name="f1u8")
        f1c1_sb = cp.tile([128, 4], F32, name="f1c1")
        nc.sync.dma_start(f1c1_sb[:], f1c1[:, :])
        f1c2_sb = cp.tile([128, 4], F32, name="f1c2")
        nc.sync.dma_start(f1c2_sb[:], f1c2[:, :])
        hr_sb = wp.tile([128, 8, TOK], F32, name="hr")
        f1b_sb = cp.tile([128, 32], F32, name="f1b")
        f1v_sb = wp.tile([128, 4, I], FP8, name="f1v8")
        f2u_sb = wp.tile([128, 32, RFC], FP8, name="f2u8")
        f2v_sb = wp.tile([128, 4, D], FP8, name="f2v8")
        f2b_sb = cp.tile([128, 8], F32, name="f2b")

        # small constants
        ones2 = cp.tile([128, 2, DH], FP8, name="ones2")
        nc.vector.memset(ones2[:], 1.0)
        ones128b = cp.tile([1, 128], BF16, name="ones128b")
        nc.vector.memset(ones128b[:], 1.0)
        eights64 = cp.tile([1, DH], BF16, name="eights64")
        nc.vector.memset(eights64[:], 8.0)
        c32 = cp.tile([128, 1], F32, name="c32")
        nc.vector.memset(c32[:], 1.0 / 32.0)
        eps1 = cp.tile([1, 1], F32, name="eps1")
        nc.vector.memset(eps1[:], LN_EPS)

        # persistent activation tiles
        hT = wp.tile([128, 8, TOK], F32, name="hT")
        hbf = wp.tile([128, 8, TOK], FP8, name="hbf")
        QT = [wp.tile([128, S], BF16, name=f"QT{f}") for f in range(2)]
        KT = [wp.tile([128, S], BF16, name=f"KT{f}") for f in range(2)]
        VA = [wp.tile([128, 16, DH + 32], FP8, name=f"VA{h}")
              for h in range(HPC)]
        Yn = wp.tile([128, 2, S], FP8, name="Yn")
        rk = [wpA.tile([128, S], BF16, name=f"rk{p}") for p in range(3)]
        rbs = [wpA.tile([128, 512], BF16, name=f"rbs{t}") for t in range(4)]
        sbs = [wpA.tile([128, 512], BF16, name=f"sbs{t}") for t in range(4)]
        a1 = wp.tile([128, 4, TOK], FP8, name="a1")
        h1 = wp.tile([128, 32, TOK], FP8, name="h1")

        rs_in = [dramp.tile([4 * 128, 512], BF16, name=f"rs_in{q}")
                 for q in range(4)]
        rs_out = [dramp.tile([128, 512], BF16, name=f"rs_out{q}")
                  for q in range(4)]

        # first ACT instruction pins the sqrt table set for phase A
        warm = cp.tile([1, 1], F32, name="warm")
        nc.scalar.activation(warm[:], eps1[:], AF.Sqrt)

        # ---------- Phase A/B/C: LN1 stats + QKV (per 512-tok chunk) -----
        phA = ExitStack()
        sqp = phA.enter_context(tc.tile_pool(name="sq", bufs=2))
        stp = phA.enter_context(tc.tile_pool(name="lnsm", bufs=2))
        wps = phA.enter_context(
            tc.tile_pool(name="wps", bufs=1, space="PSUM"))
        bcps = phA.enter_context(
            tc.tile_pool(name="bcps", bufs=1, space="PSUM"))

        sq2t = {}

        def issue_sq(tck):
            sl = slice(tck * 512, (tck + 1) * 512)
            sq2 = sqp.tile([128, 8, 512], FP8, name="sq2")
            nc.scalar.activation(sq2[:], hb[:, :, sl], AF.Square)
            sq2t[tck] = sq2

        def issue_abc(tck):
            sl = slice(tck * 512, (tck + 1) * 512)
            # QKV stage 1 (PE, fp8 DR); no LN dependence
            s1t = []
            for p in range(3):
                ps = wps.tile([128, 512], F32, name="s1", bufs=3)
                for fc in range(0, 8, 2):
                    nc.tensor.matmul(ps[:], u_sb[p][:, fc:fc + 2, :],
                                     hb[:, fc:fc + 2, sl],
                                     start=(fc == 0), stop=(fc == 6),
                                     perf_mode=DR)
                s1t.append(ps)
            # LN1 stats: sum/sumsq matmuls, Sqrt + fast recip
            sq2 = sq2t.pop(tck)
            sum_ps = wps.tile([DH, 512], F32, name="s2", bufs=2)
            sq_ps = wps.tile([DH, 512], F32, name="s2", bufs=2)
            for fc in range(0, 8, 2):
                nc.tensor.matmul(sum_ps[:], ones2[:],
                                 hb[:, fc:fc + 2, sl],
                                 start=(fc == 0), stop=(fc == 6),
                                 perf_mode=DR)
                nc.tensor.matmul(sq_ps[:], ones2[:], sq2[:, fc:fc + 2, :],
                                 start=(fc == 0), stop=(fc == 6),
                                 perf_mode=DR)
            mu = stp.tile([1, 512], F32, name="mu")
            nc.vector.tensor_scalar(mu[:], sum_ps[0:1, :], 1.0 / D,
                                    None, ALU.mult)
            t1 = stp.tile([1, 512], F32, name="t1")
            nc.vector.tensor_mul(t1[:], sum_ps[0:1, :], mu[:])
            dvar = stp.tile([1, 512], F32, name="dvar")
            nc.vector.tensor_sub(dvar[:], sq_ps[0:1, :], t1[:])
            sdev = stp.tile([1, 512], F32, name="sdev")
            nc.scalar.activation(sdev[:], dvar[:], AF.Sqrt,
                                 bias=eps1[:], scale=1.0 / D)
            rr = stp.tile([1, 512], F32, name="rr")
            nc.vector.reciprocal_approx_fast(rr[:], sdev[:])
            rrow = stp.tile([1, 512], BF16, name="rrow")
            nc.vector.tensor_copy(rrow[:], rr[:])
            srow = stp.tile([1, 512], BF16, name="srow")
            nc.vector.tensor_mul(srow[:], mu[:], rr[:])
            rb_ps = bcps.tile([128, 512], F32, name="rb")
            nc.tensor.matmul(rb_ps[:], ones128b[:], rrow[:],
                             start=True, stop=True)
            nc.vector.tensor_copy(rbs[tck][:], rb_ps[:])
            sb_ps = bcps.tile([128, 512], F32, name="sb")
            nc.tensor.matmul(sb_ps[:], ones128b[:], srow[:],
                             start=True, stop=True)
            nc.vector.tensor_copy(sbs[tck][:], sb_ps[:])
            # LN correction onto the rank representation
            for p in range(3):
                t = stp.tile([128, 512], BF16, name="t")
                nc.vector.tensor_mul(t[:], s1t[p][:], rbs[tck][:])
                nc.vector.scalar_tensor_tensor(
                    rk[p][:, sl], sbs[tck][:], uc_sb[p][:], t[:],
                    ALU.mult, ALU.add)
            # QKV stage 2 (block-diag head pairs); Q/K bias on ACT
            for p, dst, bias in ((0, QT, b2q_sb), (1, KT, b2k_sb)):
                for f in range(2):
                    fp64 = slice(64 * f, 64 * f + 64)
                    ps = wps.tile([128, 512], F32, name="s2", bufs=2)
                    nc.tensor.matmul(ps[:], v2_sb[p][fp64, f, :],
                                     rk[p][fp64, sl],
                                     start=True, stop=True)
                    nc.scalar.activation(dst[f][:, sl], ps[:], AF.Identity,
                                         bias=bias[:, f:f + 1])
            for f in range(2):
                fp64 = slice(64 * f, 64 * f + 64)
                kt0 = tck * 4
                ps = wps.tile([128, 4, 128], F32, name="s2", bufs=2)
                for kk in range(4):
                    kt = kt0 + kk
                    nc.tensor.matmul(
                        ps[:, kk, :], rk[2][fp64, kt * 128:(kt + 1) * 128],
                        v2_sb[2][fp64, f, :], start=True, stop=True)
                for j in range(2):
                    h = 2 * f + j
                    nc.vector.tensor_tensor(
                        VA[h][:, kt0:kt0 + 4, 0:DH],
                        ps[:, :, j * DH:(j + 1) * DH],
                        bv_sb[:, :, h * DH:h * DH + DH], ALU.add)

        for h in range(HPC):
            nc.vector.memset(VA[h][:, :, DH:DH + 1], 8.0)
            nc.vector.memset(VA[h][:, :, DH + 1:DH + 32], 0.0)
        issue_sq(0)
        issue_sq(1)
        issue_abc(0)
        nc.sync.dma_start(ovT_sb[:], ovT8[:, :, :])
        nc.sync.dma_start(f1u_sb[:], f1u8[:, :, :])
        nc.sync.dma_start(hr_sb[:], hr[:, :, :])
        nc.sync.dma_start(f1b_sb[:], f1b[:, :])
        issue_sq(2)
        issue_abc(1)
        nc.sync.dma_start(f1v_sb[:], f1v8[:, :, :])
        nc.sync.dma_start(f2u_sb[:], f2u8[:, :, :])
        issue_sq(3)
        issue_abc(2)
        nc.sync.dma_start(f2v_sb[:], f2v8[:, :, :])
        nc.sync.dma_start(f2b_sb[:], f2b[:, :])
        issue_abc(3)

        phA.close()
        earlyA.close()

        # ---------- attention + per-qc RS + paired local out/LN2 ------
        phD = ExitStack()      # whole tail phase
        mps = phD.enter_context(
            tc.tile_pool(name="mps", bufs=2, space="PSUM"))
        ptp = phD.enter_context(tc.tile_pool(name="pt", bufs=10))
        epi = phD.enter_context(tc.tile_pool(name="epi", bufs=2))
        stp2 = phD.enter_context(tc.tile_pool(name="st", bufs=3))
        o1p = phD.enter_context(tc.tile_pool(name="o1p", bufs=2))
        rsp = phD.enter_context(tc.tile_pool(name="rsp", bufs=4))
        gp = phD.enter_context(tc.tile_pool(name="gp", bufs=2))
        phAt = ExitStack()     # attention-only psum pools
        scps = phAt.enter_context(
            tc.tile_pool(name="scps", bufs=2, space="PSUM"))
        pvps = phAt.enter_context(
            tc.tile_pool(name="pvps", bufs=2, space="PSUM"))

        def issue_head(h, qc):
            """scores (streamed), exp, then PVs for one (head, q-chunk).

            All score matmuls are issued before any PV so the PE never
            stalls on the exp chain mid-head (keeps the HAM clock warm)."""
            rho, f = h % 2, h // 2
            pp = slice(64 * rho, 64 * rho + 64)
            q0 = qc * 512
            ntile = 4 * qc + 4
            pv = pvps.tile([DH + 32, 512], F32, name="pv")
            pts = []
            for t0 in range(0, ntile, 2):
                diag = t0 >= 4 * qc
                sc = scps.tile([128, 2, 512], F32, name="sc")
                pt = ptp.tile([128, 2, 512], FP8, name="p8")
                if not diag:
                    for j in range(2):
                        nc.tensor.matmul(
                            sc[:, j, :],
                            KT[f][pp, (t0 + j) * 128:(t0 + j + 1) * 128],
                            QT[f][pp, q0:q0 + 512],
                            start=True, stop=True)
                    nc.scalar.activation(pt[:], sc[:], AF.Exp,
                                         scale=0.125)
                else:
                    for j in range(2):
                        c0 = (t0 + j - 4 * qc) * 128
                        nc.tensor.matmul(
                            sc[:, j, c0:],
                            KT[f][pp, (t0 + j) * 128:(t0 + j + 1) * 128],
                            QT[f][pp, q0 + c0:q0 + 512],
                            start=True, stop=True)
                        nc.vector.tensor_tensor(
                            sc[:, j, c0:c0 + 128], sc[:, j, c0:c0 + 128],
                            tri_sb[:], ALU.add)
                        if c0 > 0:
                            nc.vector.memset(pt[:, j, 0:c0], 0.0)
                        nc.scalar.activation(pt[:, j, c0:],
                                             sc[:, j, c0:], AF.Exp,
                                             scale=0.125)
                pts.append(pt)
            for i, pt in enumerate(pts):
                t0 = 2 * i
                nc.tensor.matmul(pv[:], VA[h][:, t0:t0 + 2, :],
                                 pt[:], start=(t0 == 0),
                                 stop=(t0 == ntile - 2), perf_mode=DR)
            return pv

        def issue_epilogue(h, qc, pv):
            rho, f = h % 2, h // 2
            q0 = qc * 512
            den = epi.tile([1, 512], F32, name="den")
            nc.vector.tensor_copy(den[:], pv[DH:DH + 1, :])
            rec = epi.tile([1, 512], F32, name="rec")
            nc.vector.reciprocal_approx_fast(rec[:], den[:])
            rec8 = epi.tile([1, 512], BF16, name="rec8")
            nc.gpsimd.tensor_copy(rec8[:], rec[:])
            rb_ps = mps.tile([DH, 512], F32, name="m")
            nc.tensor.matmul(rb_ps[:], eights64[:], rec8[:],
                             start=True, stop=True)
            rb_sb = epi.tile([DH, 512], BF16, name="rbsb")
            nc.vector.tensor_copy(rb_sb[:], rb_ps[:])
            nc.vector.tensor_mul(Yn[64 * rho:64 * rho + 64, f,
                                    q0:q0 + 512],
                                 pv[0:DH, :], rb_sb[:])

        def issue_attn(qc):
            pend = None
            for h in range(HPC):
                pv = issue_head(h, qc)
                if pend is not None:
                    issue_epilogue(h - 1, qc, pend)
                pend = pv
            issue_epilogue(HPC - 1, qc, pend)

        def issue_rs(qc):
            """out-proj stage-1 partials ([tok, rank]) + ReduceScatter."""
            for tb in range(4):
                t0 = qc * 512 + tb * 128
                ps = mps.tile([128, 512], F32, name="m")
                nc.tensor.matmul(
                    ps[:], Yn[:, :, t0:t0 + 128], ouT_sb[:, :, :],
                    start=True, stop=True, perf_mode=DR)
                st = rsp.tile([128, 512], BF16, name="st")
                nc.vector.tensor_copy(st[:], ps[:])
                nc.sync.dma_start(
                    rs_in[qc][tb * 128:(tb + 1) * 128, :], st[:])
            nc.gpsimd.collective_compute(
                "ReduceScatter", ALU.add, replica_groups=RG,
                ins=[rs_in[qc].opt()], outs=[rs_out[qc].opt()])

        def issue_down_pair(pr):
            """out-proj s2 + residual + LN2 + fc1 s1 for 2 query chunks."""
            tsl = slice(pr * 256, (pr + 1) * 256)
            ob = o1p.tile([128, 4, 256], FP8, name="o1")
            for half in range(2):
                qc = 2 * pr + half
                g = gp.tile([128, 512], BF16, name="g")
                nc.sync.dma_start(g[:], rs_out[qc][:, :])
                for rc in range(4):
                    tp = mps.tile([128, 128], BF16, name="m")
                    nc.tensor.matmul(tp[:], g[:, rc * 128:(rc + 1) * 128],
                                     id_sb[:], start=True, stop=True,
                                     is_transpose=True)
                    nc.vector.tensor_scalar(
                        ob[:, rc, half * 128:(half + 1) * 128], tp[:],
                        1.0 / 256.0, None, ALU.mult)
            # out-proj stage 2 + residual -> hT, hbf
            for g4 in range(4):
                o2ps = mps.tile([128, 2, 256], F32, name="m")
                for k in range(2):
                    ft = g4 * 2 + k
                    for rc in range(0, 4, 2):
                        nc.tensor.matmul(
                            o2ps[:, k, :],
                            ovT_sb[:, rc:rc + 2, ft * 128:(ft + 1) * 128],
                            ob[:, rc:rc + 2, :],
                            start=(rc == 0), stop=(rc == 2), perf_mode=DR)
                nc.vector.scalar_tensor_tensor(
                    hT[:, g4 * 2:g4 * 2 + 2, tsl], o2ps[:], c32[:],
                    hr_sb[:, g4 * 2:g4 * 2 + 2, tsl],
                    ALU.mult, ALU.add)
            nc.vector.tensor_copy(hbf[:, :, tsl], hT[:, :, tsl])
            sq8 = stp2.tile([128, 8, 256], FP8, name="sq8")
            nc.scalar.activation(sq8[:], hbf[:, :, tsl], AF.Square)
            # LN2 stats + rstd chain
            sms = mps.tile([DH, 2, 256], F32, name="m")
            for fc in range(0, 8, 2):
                nc.tensor.matmul(sms[:, 0, :], ones2[:],
                                 hbf[:, fc:fc + 2, tsl],
                                 start=(fc == 0), stop=(fc == 6),
                                 perf_mode=DR)
                nc.tensor.matmul(sms[:, 1, :], ones2[:],
                                 sq8[:, fc:fc + 2, :],
                                 start=(fc == 0), stop=(fc == 6),
                                 perf_mode=DR)
            mu = stp2.tile([1, 256], F32, name="mu2")
            nc.vector.tensor_scalar(mu[:], sms[0:1, 0, :], 1.0 / D,
                                    None, ALU.mult)
            t1 = stp2.tile([1, 256], F32, name="t12")
            nc.vector.tensor_mul(t1[:], sms[0:1, 0, :], mu[:])
            dvar = stp2.tile([1, 256], F32, name="dvar2")
            nc.vector.tensor_sub(dvar[:], sms[0:1, 1, :], t1[:])
            sdev = stp2.tile([1, 256], F32, name="sdev2")
            nc.scalar.activation(sdev[:], dvar[:], AF.Sqrt,
                                 bias=eps1[:], scale=1.0 / D)
            rr = stp2.tile([1, 256], F32, name="rr2")
            nc.vector.reciprocal_approx_fast(rr[:], sdev[:])
            rrow = stp2.tile([1, 256], BF16, name="rrow2")
            nc.vector.tensor_copy(rrow[:], rr[:])
            srow = stp2.tile([1, 256], BF16, name="srow2")
            nc.vector.tensor_mul(srow[:], mu[:], rr[:])
            bc = mps.tile([128, 2, 256], F32, name="m")
            nc.tensor.matmul(bc[:, 0, :], ones128b[:], rrow[:],
                             start=True, stop=True)
            nc.tensor.matmul(bc[:, 1, :], ones128b[:], srow[:],
                             start=True, stop=True)
            rbs2 = stp2.tile([128, 2, 256], BF16, name="rbs2")
            nc.vector.tensor_copy(rbs2[:], bc[:])
            # fc1 stage 1 on raw hbf + folded LN2 correction
            for rg in range(2):
                f1ps = mps.tile([128, 2, 256], F32, name="m")
                for k in range(2):
                    rc = rg * 2 + k
                    for fc in range(0, 8, 2):
                        nc.tensor.matmul(
                            f1ps[:, k, :],
                            f1u_sb[:, fc:fc + 2, rc * 128:(rc + 1) * 128],
                            hbf[:, fc:fc + 2, tsl],
                            start=(fc == 0), stop=(fc == 6), perf_mode=DR)
                for k in range(2):
                    rc = rg * 2 + k
                    t = stp2.tile([128, 256], BF16, name="tf1")
                    nc.vector.tensor_mul(t[:], f1ps[:, k, :], rbs2[:, 0, :])
                    u = stp2.tile([128, 256], F32, name="uf1")
                    nc.vector.scalar_tensor_tensor(
                        u[:], rbs2[:, 1, :], f1c1_sb[:, rc:rc + 1], t[:],
                        ALU.mult, ALU.subtract)
                    nc.vector.tensor_scalar(
                        a1[:, rc, tsl], u[:], -1.0 / 32.0,
                        f1c2_sb[:, rc:rc + 1], ALU.mult, ALU.add)

        ffn_pools = {}

        def open_ffn_pools():
            ffn_pools["f1"] = phD.enter_context(
                tc.tile_pool(name="f1ps2", bufs=2, space="PSUM"))
            ffn_pools["acc"] = phD.enter_context(
                tc.tile_pool(name="f2acc", bufs=2, space="PSUM"))
            ffn_pools["o"] = phD.enter_context(
                tc.tile_pool(name="f2o", bufs=1, space="PSUM"))
            ffn_pools["out"] = phD.enter_context(
                tc.tile_pool(name="outp", bufs=2))

        def issue_fc1_half(half):
            """fc1 s2 + gelu for one 256-token half (fills the RS window)."""
            hsl = slice(half * 256, half * 256 + 256)
            for it in range(32):
                ps = ffn_pools["f1"].tile([128, 256], F32, name="f1s2")
                for rc in range(0, 4, 2):
                    nc.tensor.matmul(
                        ps[:], f1v_sb[:, rc:rc + 2, it * 128:(it + 1) * 128],
                        a1[:, rc:rc + 2, hsl], start=(rc == 0),
                        stop=(rc == 2), perf_mode=DR)
                nc.scalar.activation(h1[:, it, hsl], ps[:], AF.Gelu,
                                     bias=f1b_sb[:, it:it + 1],
                                     scale=1.0 / 16.0)

        def issue_ffn():
            """fc2 (full width) + residual + output."""
            a2 = ffn_pools["out"].tile([128, 4, TOK], FP8, name="a2")
            for rt in range(4):
                acc = ffn_pools["acc"].tile([128, TOK], F32, name="acc")
                for ic in range(0, 32, 2):
                    nc.tensor.matmul(
                        acc[:], f2u_sb[:, ic:ic + 2, rt * 128:(rt + 1) * 128],
                        h1[:, ic:ic + 2, :], start=(ic == 0),
                        stop=(ic == 30), perf_mode=DR)
                nc.vector.tensor_scalar(a2[:, rt, :], acc[:], 1.0 / 32.0,
                                        None, ALU.mult)
            for g4 in range(4):
                ops = ffn_pools["o"].tile([128, 2, TOK], F32, name="f2o")
                for k in range(2):
                    ft = g4 * 2 + k
                    for rc in range(0, 4, 2):
                        nc.tensor.matmul(
                            ops[:, k, :],
                            f2v_sb[:, rc:rc + 2, ft * 128:(ft + 1) * 128],
                            a2[:, rc:rc + 2, :],
                            start=(rc == 0), stop=(rc == 2), perf_mode=DR)
                ot = ffn_pools["out"].tile([128, 2, TOK], F32, name="o")
                for k in range(2):
                    ft = g4 * 2 + k
                    nc.vector.tensor_scalar(ot[:, k, :], ops[:, k, :],
                                            c32[:], f2b_sb[:, ft:ft + 1],
                                            ALU.mult, ALU.add)
                nc.vector.tensor_tensor(
                    ot[:], ot[:], hT[:, g4 * 2:g4 * 2 + 2, :], ALU.add)
                nc.sync.dma_start(out_t[:, g4 * 2:g4 * 2 + 2, :], ot[:])

        issue_attn(0)
        issue_rs(0)
        issue_attn(1)
        issue_rs(1)
        issue_attn(2)
        issue_rs(2)
        issue_attn(3)
        issue_down_pair(0)
        issue_rs(3)
        phAt.close()
        open_ffn_pools()
        issue_fc1_half(0)
        issue_down_pair(1)
        issue_fc1_half(1)
        issue_ffn()
        phD.close()

    nc.compile()
    return nc


def _q8(x, scale):
    return np.clip(np.asarray(x, np.float32) * scale,
                   -448.0, 448.0).astype(NF8)


def _chunk(x, nch):
    """[nch*128, M] -> [128, nch, M] partition-major layout."""
    m = x.shape[1]
    return np.ascontiguousarray(
        x.reshape(nch, 128, m).transpose(1, 0, 2))


def _prep_inputs(inputs):
    bf = ml_dtypes.bfloat16
    hs = np.asarray(inputs["hidden_states"], np.float32)
    g1 = np.asarray(inputs["ln1_g"], np.float32)
    b1 = np.asarray(inputs["ln1_b"], np.float32)
    g1s = np.where(g1 == 0.0, 1.0, g1)
    g2 = np.asarray(inputs["ln2_g"], np.float32)
    b2 = np.asarray(inputs["ln2_b"], np.float32)

    tri = np.where(np.triu(np.ones((128, 128), np.float32)) > 0, 0.0,
                   -240.0).astype(bf)
    oU = np.asarray(inputs["out_U"], np.float32)

    f1U = np.asarray(inputs["fc1_U"], np.float32)
    f1u_eff = _q8(f1U * g2[:, None], 32.0)
    f1c1 = f1u_eff.astype(np.float32).sum(0)            # [512]
    f1c2 = (f1U.T @ b2)                                  # [512]

    shared = {
        "trineg": tri,
        "ident": np.eye(128, dtype=np.float32).astype(bf),
        "f1b": np.ascontiguousarray(
            np.asarray(inputs["fc1_b"], np.float32).reshape(32, 128).T),
        "f2b": np.ascontiguousarray(
            np.asarray(inputs["fc2_b"], np.float32).reshape(8, 128).T),
        "ovT8": _chunk(_q8(inputs["out_V"], 32.0), 4),
        "f1u8": _chunk(f1u_eff, 8),
        "f1c1": np.ascontiguousarray(f1c1.reshape(4, 128).T),
        "f1c2": np.ascontiguousarray(f1c2.reshape(4, 128).T),
        "f1v8": _chunk(_q8(inputs["fc1_V"], 16.0), 4),
        "f2u8": _chunk(_q8(inputs["fc2_U"], 32.0), 32),
        "f2v8": _chunk(_q8(inputs["fc2_V"], 32.0), 4),
    }

    qU, kU, vU = (np.asarray(inputs[k], np.float32)
                  for k in ("q_U", "k_U", "v_U"))
    qV, kV, vV = (np.asarray(inputs[k], np.float32)
                  for k in ("q_V", "k_V", "v_V"))
    qb, kb, vb = (np.asarray(inputs[k], np.float32)
                  for k in ("q_b", "k_b", "v_b"))
    ob = np.asarray(inputs["out_b"], np.float32)

    in_maps = []
    for c in range(NCORE):
        b, g = c // 4, c % 4
        hsel = slice(4 * g, 4 * g + 4)
        m = dict(shared)
        m["hb8"] = _chunk(_q8(hs[b].T, 1.0), 8)
        toks = np.concatenate(
            [np.arange(qc * 512 + g * 128, qc * 512 + g * 128 + 128)
             for qc in range(4)])
        hrm = hs[b, toks, :].T + ob[:, None]
        m["hr"] = _chunk(hrm, 8).astype(np.float32)
        for p, (U, V, bias) in enumerate(((qU, qV, qb), (kU, kV, kb),
                                          (vU, vV, vb))):
            ue = U[:, hsel, :].reshape(D, HPC * R) * g1[:, None]
            ue_q = _q8(ue, 32.0)
            m[f"u8{p}"] = _chunk(ue_q, 8)
            ue_f = ue_q.astype(np.float32)
            m[f"uc{p}"] = np.ascontiguousarray(
                -ue_f.sum(0)[:, None]).astype(np.float32)
            Vh = V[hsel]                              # [HPC, R, DH]
            scl = (8.0 / 32.0) if p == 2 else (1.0 / 32.0)
            bd = np.zeros((128, 2, 128), np.float32)
            for f in range(2):
                bd[64 * f:64 * f + 32, f, 0:DH] = Vh[2 * f] * scl
                bd[64 * f + 32:64 * f + 64, f, DH:2 * DH] = \
                    Vh[2 * f + 1] * scl
            m[f"v2{p}"] = bd.astype(bf)
            Ut = ue_f.reshape(D, HPC, R) / (32.0 * g1s[:, None, None])
            bcor = np.einsum('d,dhr,hre->he', b1, Ut, Vh) + bias[hsel]
            if p < 2:
                b2p = np.zeros((128, 2), np.float32)
                for f in range(2):
                    b2p[0:64, f] = bcor[2 * f]
                    b2p[64:128, f] = bcor[2 * f + 1]
                m["b2q" if p == 0 else "b2k"] = b2p
            else:
                bvb = np.broadcast_to((8.0 * bcor).reshape(1, 1, HPC * DH),
                                      (128, 4, HPC * DH))
                m["bv128"] = np.ascontiguousarray(bvb).astype(np.float32)
        m["ouT8"] = _chunk(_q8(oU[256 * g:256 * (g + 1), :], 32.0), 2)
        in_maps.append(m)
    return in_maps


def kernel(trace=False, tmpdir=None, **inputs):
    if "nc" not in _cache:
        _cache["nc"] = _build_program()
    nc = _cache["nc"]
    in_maps = _prep_inputs(inputs)
    res = bass_utils.run_bass_kernel_spmd(
        nc, in_maps, core_ids=list(range(NCORE)), trace=trace,
        tmpdir=tmpdir)
    out = np.zeros((B, S, D), np.float32)
    for c in range(NCORE):
        b, g = c // 4, c % 4
        toks = np.concatenate(
            [np.arange(qc * 512 + g * 128, qc * 512 + g * 128 + 128)
             for qc in range(4)])
        r = res.results[c]["out_t"]          # [128, 8, TOK]
        out[b, toks, :] = r.transpose(1, 0, 2).reshape(D, TOK).T
    if trace:
        return out, res
    return out
